# revision 33
# baseline (speedup 1.0000x reference)
"""Trainium2 Bass kernel for nn_Net_91164975824989.

Math: the line-MLP consumes binary spike vectors s in {0,1}^3, so
MLP+softmax collapses to an 8-entry LUT; softmax over 2 outputs sums
to 1 => out[:,0] = 150 - out[:,1].  The LUT is expanded into a
multilinear polynomial over the spike bits, so per sample we only need
33 monomial sums accumulated over the 25 LIF timesteps:
  - 9 per-cell spike time-sums
  - 18 within-line pair products   (rows + cols of the 3x3 grid)
  - 6 within-line triple products
followed by a 33-weight projection (weights derived on host from the
tiny MLP weights, float64 -- O(1) work independent of batch).

Device mapping (pure data-parallel over 8 cores, 4096 samples/core):
  - layout [128 partitions, 9 cells, 32 samples] per LIF tile
  - LIF recurrence with z-transform: z = beta*z - spk, spk = (z > tau),
    tau = 1 - x/(1-beta)  (2 DVE ops/step instead of 3)
  - spikes stored bf16 (exact 0/1); pair/triple products via 6 strided
    DVE tensor_tensor ops per t-chunk
  - Sum over t on the TensorEngine: identity-weight matmuls
    accumulating into PSUM (exact integer counts in fp32)
  - epilogue: weighted per-section muls straight out of PSUM on the
    DVE, one X-axis reduce, out[:,0] = 150 - out[:,1]
Modeled (TimelineSim cost model) single-core makespan: ~44 us.
"""

import numpy as np

B = 32768
N_CORES = 8
B_CORE = B // N_CORES          # 4096
P = 128                        # partitions
SPP = B_CORE // P              # 32 samples per partition
C = 9                          # cells
T = 25                         # timesteps
NF = 33                        # features
BETA = 0.95
TGROUP = 5                     # timesteps per product/matmul chunk

_STATE: dict = {}


def _host_coeffs(W1, b1, W2, b2, W3, b3, W4, b4):
    """8-entry LUT of the line-MLP p1 output -> multilinear coeffs ->
    33 feature weights + constant term. All float64."""
    W1, b1, W2, b2, W3, b3, W4, b4 = [
        np.asarray(a, np.float64) for a in (W1, b1, W2, b2, W3, b3, W4, b4)
    ]

    def mlp_p1(s):
        h = np.maximum(W1 @ s + b1, 0)
        h = np.maximum(W2 @ h + b2, 0)
        h = np.maximum(W3 @ h + b3, 0)
        h = np.maximum(W4 @ h + b4, 0)
        e = np.exp(h - h.max())
        return e[1] / e.sum()

    u = np.zeros(8)
    for code in range(8):
        s = np.array([(code >> j) & 1 for j in range(3)], np.float64)
        u[code] = mlp_p1(s)

    # Moebius transform: u(s) = sum_m c[m] * prod_{j in m} s_j
    c = np.zeros(8)
    for m in range(8):
        for mp in range(8):
            if (mp & m) == mp:
                c[m] += (-1) ** bin(m ^ mp).count("1") * u[mp]

    c_s = [c[1], c[2], c[4]]
    c01, c02, c12 = c[3], c[5], c[6]
    c012 = c[7]

    w = np.zeros(NF)
    # f 0..8: per-cell time sums; cell c=3i+j appears in row-line i at
    # position j and col-line j at position i
    for cell in range(9):
        i, j = divmod(cell, 3)
        w[cell] = c_s[j] + c_s[i]
    # f 9..14: row pairs (j, j+1), order (i, a): a=0 -> {0,1}, a=1 -> {1,2}
    w[9:15] = [c01, c12] * 3
    # f 15..17: row pairs (0, 2)
    w[15:18] = c02
    # f 18..20: row triples
    w[18:21] = c012
    # f 21..26: col pairs (cell, cell+3), cell=0..5: rows (i, i+1)
    w[21:24] = c01
    w[24:27] = c12
    # f 27..29: col pairs (cell, cell+6): rows (0, 2)
    w[27:30] = c02
    # f 30..32: col triples
    w[30:33] = c012

    k1 = 150.0 * c[0]           # constant monomial over 25 t * 6 lines
    return w, k1


def _build_module(tgroup=TGROUP, mm=True, prod=True):
    import concourse.bass as bass
    import concourse.tile as tile
    from concourse import bacc, mybir
    from contextlib import ExitStack

    f32 = mybir.dt.float32
    bf16 = mybir.dt.bfloat16
    Alu = mybir.AluOpType

    nc = bacc.Bacc("TRN2", target_bir_lowering=False, debug=False,
                   num_devices=N_CORES)

    # x separate (compute can start as soon as it lands); aux blob per
    # partition: [ w: 32*33 | consts: 2 | identity row: 128 bf16 = 64 f32 ]
    XN = SPP * C            # 288
    WN = SPP * NF           # 1056
    BLOB = WN + 2 + P // 2  # 1122
    xs = nc.declare_dram_parameter("xs", [B_CORE, C], f32, isOutput=False)
    blob = nc.declare_dram_parameter("blob", [P, BLOB], f32, isOutput=False)
    y = nc.declare_dram_parameter("y", [B_CORE, 2], f32, isOutput=True)

    with tile.TileContext(nc) as tc, ExitStack() as ctx:
        pool = ctx.enter_context(tc.tile_pool(name="main", bufs=1))
        psum = ctx.enter_context(tc.tile_pool(name="psum", bufs=1, space="PSUM"))

        # ---- input DMAs (x first and separate: compute gates on it) ----
        x_raw_t = pool.tile([P, SPP, C], f32)
        nc.sync.dma_start(x_raw_t, xs.rearrange("(p s) c -> p s c", p=P))
        x_raw = x_raw_t[:, :, :]
        blob_sb = pool.tile([P, BLOB], f32)
        nc.sync.dma_start(blob_sb, blob[:, :])
        w_sb = blob_sb[:, :WN].rearrange("p (s f) -> p s f", f=NF)
        consts_sb = blob_sb[:, WN:WN + 2]
        id_sb = blob_sb[:, WN + 2:].bitcast(bf16)   # [P, 128]

        # ---- prologue: tau (layout [p, c, s]) and z init ----
        tau = pool.tile([P, C, SPP], f32)
        # tau[p,c,s] = 1 - 20*x[p,s,c]  (permuted write)
        nc.vector.tensor_scalar(
            out=tau.rearrange("p c s -> p s c"), in0=x_raw,
            scalar1=-20.0, scalar2=1.0, op0=Alu.mult, op1=Alu.add)
        z = pool.tile([P, C, SPP], f32)
        # z = beta * (tau - 1)
        nc.vector.tensor_scalar(
            out=z, in0=tau, scalar1=BETA, scalar2=BETA,
            op0=Alu.mult, op1=Alu.subtract)

        # ---- spike history + product history (bf16) ----
        sh = pool.tile([P, T, C, SPP], bf16)
        rp01 = pool.tile([P, T, 6, SPP], bf16)
        rp02 = pool.tile([P, T, 3, SPP], bf16)
        rtr = pool.tile([P, T, 3, SPP], bf16)
        cp03 = pool.tile([P, T, 6, SPP], bf16)
        cp06 = pool.tile([P, T, 3, SPP], bf16)
        ctr = pool.tile([P, T, 3, SPP], bf16)

        # PSUM accumulators
        ps_T = psum.tile([P, C, SPP], f32)
        ps_rp01 = psum.tile([P, 6, SPP], f32)
        ps_rp02 = psum.tile([P, 3, SPP], f32)
        ps_rtr = psum.tile([P, 3, SPP], f32)
        ps_cp03 = psum.tile([P, 6, SPP], f32)
        ps_cp06 = psum.tile([P, 3, SPP], f32)
        ps_ctr = psum.tile([P, 3, SPP], f32)

        sh_r = sh.rearrange("p t (i j) s -> p t i j s", i=3)
        rp01_r = rp01.rearrange("p t (i a) s -> p t i a s", i=3)

        if isinstance(tgroup, int):
            bounds = list(range(tgroup, T + 1, tgroup))
        else:
            bounds = []
            acc = 0
            for g in tgroup:
                acc += g
                bounds.append(acc)
        assert bounds[-1] == T

        # spk[0] = (mem_1 > 1) = (x > 1) == 0 always (x in [0,1))
        nc.vector.memset(sh[:, 0], 0)

        for t in range(1, T):
            if t == 1:
                # z = beta*z  (spk[0] == 0)
                nc.vector.tensor_scalar_mul(out=z, in0=z, scalar1=BETA)
            else:
                # z = beta*z - spk[t-1]
                nc.vector.scalar_tensor_tensor(
                    out=z, in0=z, scalar=BETA, in1=sh[:, t - 1],
                    op0=Alu.mult, op1=Alu.subtract)
            # spk[t] = (z > tau)
            nc.vector.tensor_tensor(out=sh[:, t], in0=z, in1=tau, op=Alu.is_gt)
            if mm:
                # spk[0] is all-zero, so the t=0 term is skipped entirely
                nc.tensor.matmul(ps_T[:], id_sb, sh[:, t],
                                 start=(t == 1), stop=(t == T - 1),
                                 skip_group_check=True)

            if (t + 1) in bounds:
                gi = bounds.index(t + 1)
                t0, t1 = (0 if gi == 0 else bounds[gi - 1]), t + 1
                # spk[0] == 0 -> its products vanish; skip t=0 entirely
                t0 = max(t0, 1)
                tsl = slice(t0, t1)
                # products for this t-chunk (DVE, bf16)
                if prod:
                    nc.vector.tensor_mul(rp01_r[:, tsl], sh_r[:, tsl, :, 0:2],
                                         sh_r[:, tsl, :, 1:3])
                    nc.vector.tensor_mul(rp02[:, tsl], sh_r[:, tsl, :, 0],
                                         sh_r[:, tsl, :, 2])
                    nc.vector.tensor_mul(rtr[:, tsl], rp01_r[:, tsl, :, 0],
                                         sh_r[:, tsl, :, 2])
                    nc.vector.tensor_mul(cp03[:, tsl], sh[:, tsl, 0:6],
                                         sh[:, tsl, 3:9])
                    nc.vector.tensor_mul(cp06[:, tsl], sh[:, tsl, 0:3],
                                         sh[:, tsl, 6:9])
                    nc.vector.tensor_mul(ctr[:, tsl], cp03[:, tsl, 0:3],
                                         sh[:, tsl, 6:9])
                # accumulate over t on PE (identity lhsT, PSUM accumulate)
                if mm:
                    for tt in range(t0, t1):
                        st = tt == 1
                        sp = tt == T - 1
                        for ps_tile, hist in (
                            (ps_rp01, rp01), (ps_rp02, rp02),
                            (ps_rtr, rtr), (ps_cp03, cp03), (ps_cp06, cp06),
                            (ps_ctr, ctr),
                        ):
                            nc.tensor.matmul(ps_tile[:], id_sb, hist[:, tt],
                                             start=st, stop=sp,
                                             skip_group_check=True)

        # ---- epilogue: weighted features straight out of PSUM ----
        fm = pool.tile([P, SPP, NF], f32)
        off = 0
        for ps_tile, nk in ((ps_T, 9), (ps_rp01, 6), (ps_rp02, 3),
                            (ps_rtr, 3), (ps_cp03, 6), (ps_cp06, 3),
                            (ps_ctr, 3)):
            nc.vector.tensor_mul(
                fm[:, :, off:off + nk].rearrange("p s f -> p f s"),
                ps_tile[:],
                w_sb[:, :, off:off + nk].rearrange("p s f -> p f s"))
            off += nk
        red = pool.tile([P, SPP], f32)
        nc.vector.tensor_reduce(out=red, in_=fm, axis=mybir.AxisListType.X,
                                op=Alu.add)

        out_t = pool.tile([P, SPP, 2], f32)
        # out1 = red + k1 ; out0 = (150 - k1) - red
        nc.vector.tensor_single_scalar(
            out=out_t[:, :, 1], in_=red, scalar=consts_sb[:, 0:1], op=Alu.add)
        nc.vector.tensor_scalar(
            out=out_t[:, :, 0], in0=red, scalar1=-1.0,
            scalar2=consts_sb[:, 1:2], op0=Alu.mult, op1=Alu.add)

        nc.sync.dma_start(y.rearrange("(p s) o -> p s o", p=P), out_t)

    nc.compile()
    return nc


def _get_module():
    if "nc" not in _STATE:
        _STATE["nc"] = _build_module()
    return _STATE["nc"]


def kernel(x, W1, b1, W2, b2, W3, b3, W4, b4, _trace=False):
    import ml_dtypes
    from concourse.bass_utils import run_bass_kernel_spmd

    w33, k1 = _host_coeffs(W1, b1, W2, b2, W3, b3, W4, b4)

    xs = np.asarray(x, np.float32).reshape(N_CORES, P, SPP * C)
    wrow = np.concatenate([np.tile(w33, SPP), [k1, 150.0 - k1]]).astype(
        np.float32)
    wk = np.tile(wrow[None, :], (P, 1))                      # [P, 1058]
    ident_f32 = np.ascontiguousarray(
        np.eye(P, dtype=ml_dtypes.bfloat16)).view(np.float32)  # [P, 64]

    nc = _get_module()
    blob = np.ascontiguousarray(np.concatenate([wk, ident_f32], axis=1))
    in_maps = [{"xs": np.ascontiguousarray(xs[i].reshape(B_CORE, C)),
                "blob": blob} for i in range(N_CORES)]
    res = run_bass_kernel_spmd(nc, in_maps, core_ids=list(range(N_CORES)),
                               trace=_trace)
    out = np.concatenate([res.results[i]["y"] for i in range(N_CORES)], axis=0)
    if _trace:
        _STATE["last_results"] = res
    return out.astype(np.float32)
